# revision 18
# baseline (speedup 1.0000x reference)
"""Causal self-attention (B=4, T=2048, C=2048, H=16, rope) on 8 trn2 cores.

Sharding: core c handles batch b = c//2 and head-group g = c%2 (8 heads).
Within a core, heads are processed in two sub-groups (5 then 3) so the
attention phase of group A overlaps the qkv phase of group B.

v2 changes vs v1:
  - all matmul operands bf16 (same PE rate as f32r, half the DMA, FWL on)
  - x kept resident in SBUF as [128, 16 ct, 2048] bf16 (8MB), loaded once
  - startup: first two weight tiles run ct-outer across 8 PSUM banks so
    matmuls start as soon as x chunk 0 + w0 land (vs waiting for all of x)
  - phase 4 pools allocated and Wp/yf DMAs issued before phase-2 group 1,
    and projection split into pass A (heads 0-6 -> bf16 partial in SBUF)
    and pass B (head 7 matmuls + DVE add + store), so only ~15us of PE
    work depends on the last head's AllGather.

  phase 1 (per sub-group): qkvT = W^T x^T via bf16 matmuls (N=512), rope
    applied on q/k straight out of PSUM (4 DVE ops, sign-vector trick),
    q/k stored bf16; v cast to bf16 and transposed to [t, d] on the PE.
    q/k/v round-trip DRAM (bf16).
  phase 2 (per head): scoresT = k_tile^T q (bf16), exp+scale fused on
    ACT -> bf16, causal mask multiply on diagonal tiles only, attn@V with
    a ones column appended to v so the softmax denominator falls out of
    the same matmul, per-partition reciprocal normalize, PE-transpose y
    back to [d, t].
  phase 3: per-head pairwise AllGather of y.
  phase 4: out[t, f-half] = y^T Wp^T in bf16, pass A/B as above.
"""
import sys

sys.path.insert(0, "/opt/trn_rl_repo")

import numpy as np
import ml_dtypes

import concourse.bass as bass
import concourse.tile as tile
from concourse import bacc, mybir
from concourse import bass_utils

F32 = mybir.dt.float32
BF16 = mybir.dt.bfloat16
AF = mybir.ActivationFunctionType
ALU = mybir.AluOpType
BF16NP = ml_dtypes.bfloat16

B, T, C = 4, 2048, 2048
NH, D = 16, 128
HL = 8              # heads per core
NCT = C // 128      # 16 c-tiles
NTT = T // 128      # 16 t-tiles
SCALE = 1.0 / np.sqrt(D)
RG = [[0, 1], [2, 3], [4, 5], [6, 7]]
GROUPS = [list(range(0, 5)), list(range(5, 8))]   # head sub-groups


def _interleave(units_a, units_b):
    """Round-robin emit closures from two lists, proportionally."""
    na, nb = len(units_a), len(units_b)
    ia = ib = 0
    while ia < na or ib < nb:
        if ib >= nb or (ia < na and ia * nb <= ib * na):
            units_a[ia]()
            ia += 1
        else:
            units_b[ib]()
            ib += 1


def _build():
    nc = bacc.Bacc("TRN2", target_bir_lowering=False, debug=False, num_devices=8)
    xT8 = nc.dram_tensor("xT8", [128, NCT, T], BF16, kind="ExternalInput").ap()
    Wall = nc.dram_tensor("Wall", [24, 128, C], BF16, kind="ExternalInput").ap()
    WpT = nc.dram_tensor("WpT", [128, NCT, C // 2], BF16, kind="ExternalInput").ap()
    cos2 = nc.dram_tensor("cos2", [128, T], F32, kind="ExternalInput").ap()
    sin1 = nc.dram_tensor("sin1", [64, T], F32, kind="ExternalInput").ap()
    sgn = nc.dram_tensor("sgn", [128, 1], F32, kind="ExternalInput").ap()
    mask4 = nc.dram_tensor("mask4", [128, 4, 512], BF16, kind="ExternalInput").ap()
    ident = nc.dram_tensor("ident", [128, 128], BF16, kind="ExternalInput").ap()
    out = nc.dram_tensor("out", [T, C // 2], BF16, kind="ExternalOutput").ap()

    with tile.TileContext(nc) as tc:
        with tc.tile_pool(name="dram", bufs=1, space="DRAM") as dram, \
             tc.tile_pool(name="const", bufs=1) as cpool:
            qk_d = [dram.tile([128, T], BF16, name=f"qk_d{i}") for i in range(16)]
            v_d = [dram.tile([128, NTT, 128], BF16, name=f"v_d{i}") for i in range(HL)]
            yg_in = [dram.tile([128, T], BF16, name=f"yg_in{h}")
                     for h in range(HL)]
            yg_out = [dram.tile([2, 128, T], BF16, name=f"yg_out{h}")
                      for h in range(HL)]

            # Const tiles: DMAs for the bigger ones are emitted after the
            # startup-critical w0/w1/x loads (see below).
            m4_sb = cpool.tile([128, 4, 512], BF16)
            id_sb = cpool.tile([128, 128], BF16)
            sg_sb = cpool.tile([128, 1], F32)
            c2_sb = cpool.tile([128, T], F32)
            s1_sb = cpool.tile([64, T], F32)

            # Pool stacks are LIFO per (space, side). Left stack: p2 pools at
            # the bottom (die after phase 2), p1 pools on top (die after
            # phase 1). Right stack: phase-4 pools, allocated mid-kernel and
            # released at the end.
            p2 = {}
            p2["qk"] = tc.alloc_tile_pool(name="p2qk", bufs=3)
            p2["va"] = tc.alloc_tile_pool(name="p2va", bufs=2)
            p2["eb"] = tc.alloc_tile_pool(name="p2eb", bufs=8)
            p2["yn"] = tc.alloc_tile_pool(name="p2yn", bufs=3)
            p2["rc"] = tc.alloc_tile_pool(name="p2rc", bufs=3)
            p2["yts"] = tc.alloc_tile_pool(name="p2yts", bufs=2)
            p2["sp"] = tc.alloc_tile_pool(name="p2sp", bufs=2, space="PSUM")
            p2["yp"] = tc.alloc_tile_pool(name="p2yp", bufs=2, space="PSUM")
            p1 = {}
            p1["w"] = tc.alloc_tile_pool(name="p1w", bufs=3)
            p1["ab"] = tc.alloc_tile_pool(name="p1ab", bufs=2)
            p1["qr"] = tc.alloc_tile_pool(name="p1qr", bufs=2)
            p1["v"] = tc.alloc_tile_pool(name="p1v", bufs=2)
            p1["ps"] = tc.alloc_tile_pool(name="p1ps", bufs=2, space="PSUM")
            p1["x"] = tc.alloc_tile_pool(name="p1x", bufs=1)

            def load_w(gi, wi):
                base = 15 * gi
                wt = p1["w"].tile([128, C], BF16, name="wt")
                (nc.sync if wi % 2 == 0 else nc.scalar).dma_start(
                    wt[:], Wall[base + wi])
                return wt

            # First two weight tiles go out before x so the startup matmuls
            # only wait on w0 + the x chunk they consume; the identity (for
            # the first v transposes) rides along on the gpsimd queue.
            wt_start = [load_w(0, 0), load_w(0, 1)]
            nc.gpsimd.dma_start(id_sb[:], ident)

            # x resident in SBUF: [128 c_lo, 16 ct, 2048 t] bf16, one DMA
            # per (ct, t-half) so the first matmuls only wait on their chunk.
            x_sb = p1["x"].tile([128, NCT, T], BF16, name="x_sb")
            engs = (nc.sync, nc.scalar, nc.gpsimd)
            for th in range(2):
                for ct in range(NCT):
                    engs[(th * NCT + ct) % 3].dma_start(
                        x_sb[:, ct:ct + 1, th * 1024:(th + 1) * 1024],
                        xT8[:, ct:ct + 1, th * 1024:(th + 1) * 1024])
                if th == 0:
                    # rope tables + sign vector: needed by the first q/k
                    # finish (~45us in); mask4 not until phase 2.
                    nc.gpsimd.dma_start(c2_sb[:], cos2)
                    nc.scalar.dma_start(s1_sb[:], sin1)
                    nc.sync.dma_start(sg_sb[:], sgn)
            nc.gpsimd.dma_start(m4_sb[:], mask4)

            # ---------- phase 1 ----------
            def wtile_info(gi, wi):
                G = GROUPS[gi]
                if wi < len(G):
                    return "v", G[wi]
                return "qk"[(wi - len(G)) % 2], G[(wi - len(G)) // 2]

            def finish_tile(gi, wi, ts, ps):
                """Consume one [128,512] qkv PSUM tile (rope or v-pack)."""
                kind, h = wtile_info(gi, wi)
                t0 = ts * 512
                if kind == "v":
                    vb = p1["v"].tile([128, 512], BF16, name="vb")
                    vt4 = p1["v"].tile([128, 4, 128], BF16, name="vt4")
                    for qq in range(4):
                        nc.scalar.copy(vb[:, qq * 128:(qq + 1) * 128],
                                       ps[:, qq * 128:(qq + 1) * 128])
                        vtp = p1["ps"].tile([128, 128], BF16, name="vtp",
                                            tag="qkvps")
                        nc.tensor.transpose(
                            vtp[:], vb[:, qq * 128:(qq + 1) * 128], id_sb[:])
                        nc.vector.tensor_copy(vt4[:, qq, :], vtp[:])
                    tt0 = t0 // 128
                    nc.scalar.dma_start(v_d[h][:, tt0:tt0 + 4, :], vt4[:])
                else:
                    a_t = p1["ab"].tile([128, 512], F32, name="a_t")
                    nc.vector.tensor_mul(
                        a_t[:], ps[:], c2_sb[:, t0:t0 + 512])
                    b_t = p1["ab"].tile([128, 512], F32, name="b_t")
                    nc.vector.tensor_mul(
                        b_t[0:64, :], ps[64:128, :], s1_sb[:, t0:t0 + 512])
                    nc.vector.tensor_mul(
                        b_t[64:128, :], ps[0:64, :], s1_sb[:, t0:t0 + 512])
                    qr = p1["qr"].tile([128, 512], BF16, name="qr")
                    nc.vector.scalar_tensor_tensor(
                        qr[:], b_t[:], sg_sb[:], a_t[:],
                        op0=ALU.mult, op1=ALU.add)
                    rt = h if kind == "q" else 8 + h
                    nc.sync.dma_start(qk_d[rt][:, t0:t0 + 512], qr[:])

            def p1_startup_unit():
                """w-tile 0 of group 0, ct-outer over 2 psums: matmuls start
                as soon as w0 + x chunk (ct=0, half 0) have landed."""
                def go():
                    pss = [p1["ps"].tile([128, 512], F32, name="qkvps")
                           for _ in range(2)]
                    for ct in range(NCT):
                        for ts in range(2):
                            nc.tensor.matmul(
                                pss[ts][:],
                                wt_start[0][:, ct * 128:(ct + 1) * 128],
                                x_sb[:, ct, ts * 512:(ts + 1) * 512],
                                start=(ct == 0), stop=(ct == NCT - 1))
                    for ts in range(2):
                        finish_tile(0, 0, ts, pss[ts])
                return go

            def p1_unit(gi, wi, ts_list=(0, 1, 2, 3), wt=None):
                def go():
                    w = wt if wt is not None else load_w(gi, wi)
                    for ts in ts_list:
                        ps = p1["ps"].tile([128, 512], F32, name="qkvps")
                        for ct in range(NCT):
                            nc.tensor.matmul(
                                ps[:], w[:, ct * 128:(ct + 1) * 128],
                                x_sb[:, ct, ts * 512:(ts + 1) * 512],
                                start=(ct == 0), stop=(ct == NCT - 1))
                        finish_tile(gi, wi, ts, ps)
                return go

            def p1_units(gi):
                if gi == 0:
                    return ([p1_startup_unit(),
                             p1_unit(0, 0, (2, 3), wt_start[0]),
                             p1_unit(0, 1, (0, 1, 2, 3), wt_start[1])] +
                            [p1_unit(0, wi) for wi in range(2, 15)])
                return [p1_unit(1, wi) for wi in range(9)]

            # ---------- phase 2 ----------
            emit_yf = {}   # set before p2_units(1) runs

            def p2_units(gi):
                G = GROUPS[gi]
                units = []
                st = {}

                def prologue(h):
                    def go():
                        qt = p2["qk"].tile([128, T], BF16, name="qt")
                        nc.sync.dma_start(qt[:], qk_d[h][:])
                        kt = p2["qk"].tile([128, T], BF16, name="kt")
                        nc.sync.dma_start(kt[:], qk_d[8 + h][:])
                        va = p2["va"].tile([128, NTT, 129], BF16, name="va")
                        nc.scalar.dma_start(va[:, :, 0:128], v_d[h][:])
                        nc.vector.memset(va[:, :, 128:129], 1.0)
                        yts = p2["yts"].tile([128, NTT, 128], BF16, name="yts")
                        st[h] = (qt, kt, va, yts, [])
                    return go

                def chunk(h, Q):
                    def go():
                        qt, kt, va, yts, ebs = st[h]
                        del ebs[:]
                        for b2 in range(2 * Q + 2):
                            sp = p2["sp"].tile([128, 2, 512], F32, name="sp")
                            for jj in range(2):
                                j = 2 * b2 + jj
                                nc.tensor.matmul(
                                    sp[:, jj, :],
                                    kt[:, j * 128:(j + 1) * 128],
                                    qt[:, Q * 512:(Q + 1) * 512],
                                    start=True, stop=True)
                            eb = p2["eb"].tile([128, 2, 512], BF16, name="eb")
                            nc.scalar.activation(
                                eb[:], sp[:], AF.Exp, scale=float(SCALE))
                            if b2 == 2 * Q:
                                nc.vector.tensor_mul(
                                    eb[:], eb[:], m4_sb[:, 0:2, :])
                            elif b2 == 2 * Q + 1:
                                nc.vector.tensor_mul(
                                    eb[:], eb[:], m4_sb[:, 2:4, :])
                            ebs.append(eb)
                        for ql in range(4):
                            qt_i = Q * 4 + ql
                            yp = p2["yp"].tile([128, 129], F32, name="yp")
                            for j in range(qt_i + 1):
                                nc.tensor.matmul(
                                    yp[:],
                                    ebs[j // 2][:, j % 2,
                                                ql * 128:(ql + 1) * 128],
                                    va[:, j, :],
                                    start=(j == 0), stop=(j == qt_i))
                            rc = p2["rc"].tile([128, 1], F32, name="rc")
                            nc.vector.reciprocal(rc[:], yp[:, 128:129])
                            yn = p2["yn"].tile([128, 128], BF16, name="yn")
                            nc.vector.tensor_scalar_mul(
                                yn[:], yp[:, 0:128], rc[:])
                            ytp = p2["yp"].tile([128, 128], BF16, name="ytp",
                                                tag="yp")
                            nc.tensor.transpose(ytp[:], yn[:], id_sb[:])
                            nc.vector.tensor_copy(yts[:, qt_i, :], ytp[:])
                    return go

                def epilogue(h):
                    def go():
                        yts = st[h][3]
                        nc.scalar.dma_start(
                            yg_in[h].rearrange("d (tt t) -> d tt t", t=128),
                            yts[:])
                        nc.gpsimd.collective_compute(
                            "AllGather", ALU.bypass,
                            ins=[yg_in[h][:].opt()], outs=[yg_out[h][:].opt()],
                            replica_groups=RG)
                        if emit_yf:
                            emit_yf["fn"](h)
                        del st[h]
                    return go

                for h in G:
                    units.append(prologue(h))
                    for Q in range(4):
                        units.append(chunk(h, Q))
                    units.append(epilogue(h))
                return units

            # ---------- emit ----------
            for u in p1_units(0):
                u()
            _interleave(p1_units(1), p2_units(0))

            # x + phase-1 pools done after p1 group 1; free them (LIFO) so
            # the phase-4 weight/y buffers can be allocated (right side) and
            # their DMAs issued while phase-2 group 1 computes.
            for key in ("x", "ps", "v", "qr", "ab", "w"):
                p1[key].release()

            p4w = tc.alloc_tile_pool(name="p4w", bufs=1, side="right")
            p4y = tc.alloc_tile_pool(name="p4y", bufs=1, side="right")
            pa_pool = tc.alloc_tile_pool(name="p4pa", bufs=1, side="right")
            ppa_pool = tc.alloc_tile_pool(name="p4psA", bufs=2, side="right",
                                          space="PSUM")
            wp = p4w.tile([128, NCT, C // 2], BF16)
            nc.gpsimd.dma_start(wp[:], WpT)
            partial = pa_pool.tile([128, NTT, 2, 512], BF16)
            yfs = {}

            def _emit_yf(h):
                yf = p4y.tile([128, 2, NTT, 128], BF16, name=f"yf{h}")
                eng = nc.sync if h % 2 == 0 else nc.scalar
                eng.dma_start(
                    yf[:], yg_out[h][:].rearrange(
                        "r d (tt t) -> d r tt t", t=128))
                yfs[h] = yf

            # group-0 heads have already gathered; group-1 heads emit their
            # yf load inside the epilogue, right after their collective.
            for h in GROUPS[0]:
                _emit_yf(h)
            emit_yf["fn"] = _emit_yf

            # ---------- phase 4 pass A0 (heads 0-3): interleaved with
            # phase-2 group 1 to fill its pipeline bubbles ----------
            def a0_unit(tt, fc):
                def go():
                    pp = ppa_pool.tile([128, 512], F32, name="ppA")
                    for i, (r, h) in enumerate(
                            [(r, h) for h in range(4) for r in range(2)]):
                        nc.tensor.matmul(
                            pp[:], yfs[h][:, r, tt, :],
                            wp[:, r * 8 + h, fc * 512:(fc + 1) * 512],
                            start=(i == 0), stop=(i == 7))
                    nc.vector.tensor_copy(partial[:, tt, fc, :], pp[:])
                return go

            a0_units = [a0_unit(tt, fc) for tt in range(NTT)
                        for fc in range(2)]
            _interleave(p2_units(1), a0_units)

            for key in ("yp", "sp", "yts", "rc", "yn", "eb", "va", "qk"):
                p2[key].release()

            # ---------- phase 4: pass A1 (heads 4-6) + pass B (head 7) ----
            ppb_pool = tc.alloc_tile_pool(name="p4psB", bufs=4, side="right",
                                          space="PSUM")
            o_pool = tc.alloc_tile_pool(name="p4o", bufs=4, side="right")
            for tt in range(NTT):
                for fc in range(2):
                    pp = ppb_pool.tile([128, 512], F32, name="ppB")
                    for i, (r, h) in enumerate(
                            [(r, h) for h in (4, 5, 6) for r in range(2)]):
                        nc.tensor.matmul(
                            pp[:], yfs[h][:, r, tt, :],
                            wp[:, r * 8 + h, fc * 512:(fc + 1) * 512],
                            start=(i == 0), stop=(i == 5))
                    nc.vector.tensor_add(
                        partial[:, tt, fc, :], pp[:], partial[:, tt, fc, :])
            for tt in range(NTT):
                for fc in range(2):
                    pp = ppb_pool.tile([128, 512], F32, name="ppB")
                    for i, r in enumerate((0, 1)):
                        nc.tensor.matmul(
                            pp[:], yfs[7][:, r, tt, :],
                            wp[:, r * 8 + 7, fc * 512:(fc + 1) * 512],
                            start=(i == 0), stop=(i == 1))
                    ob = o_pool.tile([128, 512], BF16, name="ob")
                    nc.vector.tensor_add(
                        ob[:], pp[:], partial[:, tt, fc, :])
                    (nc.sync if (tt + fc) % 2 == 0 else nc.scalar).dma_start(
                        out[tt * 128:(tt + 1) * 128,
                            fc * 512:(fc + 1) * 512], ob[:])

            o_pool.release()
            ppb_pool.release()
            ppa_pool.release()
            pa_pool.release()
            p4y.release()
            p4w.release()
    nc.compile()
    return nc


_NC = None


def _get_nc():
    global _NC
    if _NC is None:
        _NC = _build()
    return _NC


def _rope_tables():
    inv_freq = (1.0 / (10000.0 ** (np.arange(0, D, 2, dtype=np.float32) / D)))
    t = np.arange(T, dtype=np.float32)
    freqs = np.outer(t, inv_freq).astype(np.float32)      # [T, 64]
    cos = np.cos(freqs).T                                 # [64, T]
    sin = np.sin(freqs).T
    cos2 = np.concatenate([cos, cos], 0).astype(np.float32)
    sin1 = np.ascontiguousarray(sin.astype(np.float32))
    return cos2, sin1


def _tile_w(Wt):
    """[128 r, 2048 c] weight tile -> [128 c_lo, 2048 (ct r)] layout."""
    return np.ascontiguousarray(
        Wt.T.reshape(NCT, 128, 128).transpose(1, 0, 2).reshape(128, C))


def make_in_maps(x, W_attn, W_proj):
    perm = np.concatenate([np.arange(0, D, 2), np.arange(1, D, 2)])
    cos2, sin1 = _rope_tables()
    sgn = np.concatenate([-np.ones((64, 1)), np.ones((64, 1))]).astype(np.float32)
    p_i = np.arange(128)[:, None, None]
    jj_i = np.arange(4)[None, :, None]
    c_i = np.arange(512)[None, None, :]
    mask4 = (c_i >= p_i + 128 * jj_i).astype(BF16NP)

    in_maps = []
    for core in range(8):
        b, g = core // 2, core % 2
        tiles = []
        for G in GROUPS:
            for h in G:
                hg = g * HL + h
                tiles.append(_tile_w(W_attn[2 * C + hg * D:2 * C + (hg + 1) * D]))
            for h in G:
                hg = g * HL + h
                tiles.append(_tile_w(W_attn[hg * D:(hg + 1) * D][perm]))
                tiles.append(_tile_w(W_attn[C + hg * D:C + (hg + 1) * D][perm]))
        Wall = np.stack(tiles, 0).astype(BF16NP)
        WpT = np.ascontiguousarray(
            W_proj[g * (C // 2):(g + 1) * (C // 2), :].T
        ).reshape(NCT, 128, C // 2).transpose(1, 0, 2)
        xT8 = np.ascontiguousarray(
            x[b].T.reshape(NCT, 128, T).transpose(1, 0, 2)).astype(BF16NP)
        in_maps.append({
            "xT8": xT8,
            "Wall": Wall,
            "WpT": np.ascontiguousarray(WpT).astype(BF16NP),
            "cos2": cos2, "sin1": sin1, "sgn": sgn,
            "mask4": mask4, "ident": np.eye(128, dtype=BF16NP),
        })
    return in_maps


def _assemble(results):
    out = np.empty((B, T, C), dtype=np.float32)
    for core in range(8):
        b, g = core // 2, core % 2
        out[b][:, g * (C // 2):(g + 1) * (C // 2)] = results[core]["out"]
    return out


def run(x, W_attn, W_proj, **spmd_kwargs):
    nc = _get_nc()
    in_maps = make_in_maps(np.asarray(x, dtype=np.float32),
                           np.asarray(W_attn, dtype=np.float32),
                           np.asarray(W_proj, dtype=np.float32))
    res = bass_utils.run_bass_kernel_spmd(
        nc, in_maps, core_ids=list(range(8)), **spmd_kwargs)
    return _assemble(res.results), res


def kernel(x, W_attn, W_proj):
    out, _ = run(x, W_attn, W_proj)
    return out


# revision 47
# speedup vs baseline: 1.0233x; 1.0233x over previous
"""Causal self-attention (B=4, T=2048, C=2048, H=16, rope) on 8 trn2 cores.

Sharding: core c handles batch b = c//2 and head-group g = c%2 (8 heads).
Within a core, heads are processed in two sub-groups (5 then 3) so the
attention phase of group A overlaps the qkv phase of group B.

v2 changes vs v1:
  - all matmul operands bf16 (same PE rate as f32r, half the DMA, FWL on)
  - x kept resident in SBUF as [128, 16 ct, 2048] bf16 (8MB), loaded once
  - startup: first two weight tiles run ct-outer across 8 PSUM banks so
    matmuls start as soon as x chunk 0 + w0 land (vs waiting for all of x)
  - phase 4 pools allocated and Wp/yf DMAs issued before phase-2 group 1,
    and projection split into pass A (heads 0-6 -> bf16 partial in SBUF)
    and pass B (head 7 matmuls + DVE add + store), so only ~15us of PE
    work depends on the last head's AllGather.

  phase 1 (per sub-group): qkvT = W^T x^T via bf16 matmuls (N=512), rope
    applied on q/k straight out of PSUM (4 DVE ops, sign-vector trick),
    q/k stored bf16; v cast to bf16 and transposed to [t, d] on the PE.
    q/k/v round-trip DRAM (bf16).
  phase 2 (per head): scoresT = k_tile^T q (bf16), exp+scale fused on
    ACT -> bf16, causal mask multiply on diagonal tiles only, attn@V with
    a ones column appended to v so the softmax denominator falls out of
    the same matmul, per-partition reciprocal normalize, PE-transpose y
    back to [d, t].
  phase 3: per-head pairwise AllGather of y.
  phase 4: out[t, f-half] = y^T Wp^T in bf16, pass A/B as above.
"""
import sys

sys.path.insert(0, "/opt/trn_rl_repo")

import numpy as np
import ml_dtypes

import concourse.bass as bass
import concourse.tile as tile
from concourse import bacc, mybir
from concourse import bass_utils

F32 = mybir.dt.float32
BF16 = mybir.dt.bfloat16
AF = mybir.ActivationFunctionType
ALU = mybir.AluOpType
BF16NP = ml_dtypes.bfloat16

B, T, C = 4, 2048, 2048
NH, D = 16, 128
HL = 8              # heads per core
NCT = C // 128      # 16 c-tiles
NTT = T // 128      # 16 t-tiles
SCALE = 1.0 / np.sqrt(D)
RG = [[0, 1], [2, 3], [4, 5], [6, 7]]
GROUPS = [list(range(0, 5)), list(range(5, 8))]   # head sub-groups


def _interleave(units_a, units_b):
    """Round-robin emit closures from two lists, proportionally."""
    na, nb = len(units_a), len(units_b)
    ia = ib = 0
    while ia < na or ib < nb:
        if ib >= nb or (ia < na and ia * nb <= ib * na):
            units_a[ia]()
            ia += 1
        else:
            units_b[ib]()
            ib += 1


def _build():
    nc = bacc.Bacc("TRN2", target_bir_lowering=False, debug=False, num_devices=8)
    xT8 = nc.dram_tensor("xT8", [128, NCT, T], BF16, kind="ExternalInput").ap()
    Wall = nc.dram_tensor("Wall", [24, 128, C], BF16, kind="ExternalInput").ap()
    WpT = nc.dram_tensor("WpT", [128, NCT, C // 2], BF16, kind="ExternalInput").ap()
    cos2 = nc.dram_tensor("cos2", [128, T], F32, kind="ExternalInput").ap()
    sin1 = nc.dram_tensor("sin1", [64, T], F32, kind="ExternalInput").ap()
    sgn = nc.dram_tensor("sgn", [128, 1], F32, kind="ExternalInput").ap()
    mask4 = nc.dram_tensor("mask4", [128, 4, 512], BF16, kind="ExternalInput").ap()
    ident = nc.dram_tensor("ident", [128, 128], BF16, kind="ExternalInput").ap()
    out = nc.dram_tensor("out", [T, C // 2], BF16, kind="ExternalOutput").ap()

    with tile.TileContext(nc) as tc:
        with tc.tile_pool(name="dram", bufs=1, space="DRAM") as dram, \
             tc.tile_pool(name="const", bufs=1) as cpool:
            qk_d = [dram.tile([128, T], BF16, name=f"qk_d{i}") for i in range(16)]
            v_d = [dram.tile([128, NTT, 128], BF16, name=f"v_d{i}")
                   for i in range(HL)]
            y_raw = [dram.tile([128, T], BF16, name=f"y_raw{h}")
                     for h in range(HL)]
            yg_out = [dram.tile([2, 128, T], BF16, name=f"yg_out{h}")
                      for h in range(HL)]

            # Const tiles: DMAs are emitted after the startup-critical
            # w0/w1/x loads (see below). The rope tables only live through
            # phase 1, so they go in their own pool (top of the left stack,
            # released with the phase-1 pools).
            m4_sb = cpool.tile([128, 4, 512], BF16)
            id_sb = cpool.tile([128, 128], BF16)

            # Pool stacks are LIFO per (space, side). Left stack: p2 pools at
            # the bottom (die after phase 2), p1 pools on top (die after
            # phase 1). Right stack: phase-4 pools, allocated mid-kernel and
            # released at the end.
            p2 = {}
            p2["qk"] = tc.alloc_tile_pool(name="p2qk", bufs=3)
            p2["va"] = tc.alloc_tile_pool(name="p2va", bufs=4)
            p2["eb"] = tc.alloc_tile_pool(name="p2eb", bufs=8)
            p2["yst"] = tc.alloc_tile_pool(name="p2yst", bufs=2)
            p2["yn"] = tc.alloc_tile_pool(name="p2yn", bufs=3)
            p2["rc"] = tc.alloc_tile_pool(name="p2rc", bufs=3)
            p2["sp"] = tc.alloc_tile_pool(name="p2sp", bufs=2, space="PSUM")
            p2["yp"] = tc.alloc_tile_pool(name="p2yp", bufs=2, space="PSUM")
            p1 = {}
            p1["w"] = tc.alloc_tile_pool(name="p1w", bufs=3)
            p1["ab"] = tc.alloc_tile_pool(name="p1ab", bufs=2)
            p1["qr"] = tc.alloc_tile_pool(name="p1qr", bufs=2)
            p1["v"] = tc.alloc_tile_pool(name="p1v", bufs=2)
            p1["ps"] = tc.alloc_tile_pool(name="p1ps", bufs=2, space="PSUM")
            p1["x"] = tc.alloc_tile_pool(name="p1x", bufs=1)
            p1["rope"] = tc.alloc_tile_pool(name="p1rope", bufs=1)
            sg_sb = p1["rope"].tile([128, 1], F32)
            c2_sb = p1["rope"].tile([128, T], F32)
            s1_sb = p1["rope"].tile([64, T], F32)

            def load_w(gi, wi):
                base = 15 * gi
                wt = p1["w"].tile([128, C], BF16, name="wt")
                (nc.sync if wi % 2 == 0 else nc.scalar).dma_start(
                    wt[:], Wall[base + wi])
                return wt

            # First two weight tiles go out before x so the startup matmuls
            # only wait on w0 + the x chunk they consume. All startup-
            # critical DMAs ride the two hardware-DGE queues (sync/scalar);
            # the gpsimd software-DGE queue has ~tens-of-us startup latency
            # and only carries the late-needed mask.
            wt_start = [load_w(0, 0), load_w(0, 1)]
            nc.sync.dma_start(id_sb[:], ident)

            # x resident in SBUF: [128 c_lo, 16 ct, 2048 t] bf16, one DMA
            # per (ct, t-half) so the first matmuls only wait on their chunk.
            x_sb = p1["x"].tile([128, NCT, T], BF16, name="x_sb")
            engs = (nc.sync, nc.scalar)
            for th in range(2):
                for ct in range(NCT):
                    engs[ct % 2].dma_start(
                        x_sb[:, ct:ct + 1, th * 1024:(th + 1) * 1024],
                        xT8[:, ct:ct + 1, th * 1024:(th + 1) * 1024])
                if th == 0:
                    # rope tables + sign vector: needed by the first q/k
                    # finish (~45us in); mask4 not until phase 2.
                    nc.sync.dma_start(sg_sb[:], sgn)
                    nc.scalar.dma_start(c2_sb[:], cos2)
            nc.scalar.dma_start(s1_sb[:], sin1)
            nc.gpsimd.dma_start(m4_sb[:], mask4)

            # ---------- phase 1 ----------
            def wtile_info(gi, wi):
                G = GROUPS[gi]
                if wi < len(G):
                    return "v", G[wi]
                return "qk"[(wi - len(G)) % 2], G[(wi - len(G)) // 2]

            def finish_tile(gi, wi, ts, ps):
                """Consume one [128,512] qkv PSUM tile (rope or v-pack)."""
                kind, h = wtile_info(gi, wi)
                t0 = ts * 512
                if kind == "v":
                    vb = p1["v"].tile([128, 512], BF16, name="vb")
                    vt4 = p1["v"].tile([128, 4, 128], BF16, name="vt4")
                    for qq in range(4):
                        nc.scalar.copy(vb[:, qq * 128:(qq + 1) * 128],
                                       ps[:, qq * 128:(qq + 1) * 128])
                        vtp = p1["ps"].tile([128, 128], BF16, name="vtp",
                                            tag="qkvps")
                        nc.tensor.transpose(
                            vtp[:], vb[:, qq * 128:(qq + 1) * 128], id_sb[:])
                        nc.vector.tensor_copy(vt4[:, qq, :], vtp[:])
                    tt0 = t0 // 128
                    nc.scalar.dma_start(v_d[h][:, tt0:tt0 + 4, :], vt4[:])
                else:
                    a_t = p1["ab"].tile([128, 512], F32, name="a_t")
                    nc.vector.tensor_mul(
                        a_t[:], ps[:], c2_sb[:, t0:t0 + 512])
                    b_t = p1["ab"].tile([128, 512], F32, name="b_t")
                    nc.vector.tensor_mul(
                        b_t[0:64, :], ps[64:128, :], s1_sb[:, t0:t0 + 512])
                    nc.vector.tensor_mul(
                        b_t[64:128, :], ps[0:64, :], s1_sb[:, t0:t0 + 512])
                    qr = p1["qr"].tile([128, 512], BF16, name="qr")
                    nc.vector.scalar_tensor_tensor(
                        qr[:], b_t[:], sg_sb[:], a_t[:],
                        op0=ALU.mult, op1=ALU.add)
                    rt = h if kind == "q" else 8 + h
                    nc.sync.dma_start(qk_d[rt][:, t0:t0 + 512], qr[:])

            def p1_startup_unit():
                """w-tile 0 of group 0, ct-outer over 2 psums: matmuls start
                as soon as w0 + x chunk (ct=0, half 0) have landed."""
                def go():
                    pss = [p1["ps"].tile([128, 512], F32, name="qkvps")
                           for _ in range(2)]
                    for ct in range(NCT):
                        for ts in range(2):
                            nc.tensor.matmul(
                                pss[ts][:],
                                wt_start[0][:, ct * 128:(ct + 1) * 128],
                                x_sb[:, ct, ts * 512:(ts + 1) * 512],
                                start=(ct == 0), stop=(ct == NCT - 1))
                    for ts in range(2):
                        finish_tile(0, 0, ts, pss[ts])
                return go

            def p1_unit(gi, wi, ts_list=(0, 1, 2, 3), wt=None):
                def go():
                    w = wt if wt is not None else load_w(gi, wi)
                    for ts in ts_list:
                        ps = p1["ps"].tile([128, 512], F32, name="qkvps")
                        for ct in range(NCT):
                            nc.tensor.matmul(
                                ps[:], w[:, ct * 128:(ct + 1) * 128],
                                x_sb[:, ct, ts * 512:(ts + 1) * 512],
                                start=(ct == 0), stop=(ct == NCT - 1))
                        finish_tile(gi, wi, ts, ps)
                return go

            def p1_units(gi):
                if gi == 0:
                    return ([p1_startup_unit(),
                             p1_unit(0, 0, (2, 3), wt_start[0]),
                             p1_unit(0, 1, (0, 1, 2, 3), wt_start[1])] +
                            [p1_unit(0, wi) for wi in range(2, 15)])
                return [p1_unit(1, wi) for wi in range(9)]

            # ---------- phase 2 ----------
            emit_yf = {}   # set before p2_units(1) runs

            def p2_units(gi):
                G = GROUPS[gi]
                units = []
                st = {}

                def prologue(h):
                    def go():
                        qt = p2["qk"].tile([128, T], BF16, name="qt")
                        nc.sync.dma_start(qt[:], qk_d[h][:])
                        kt = p2["qk"].tile([128, T], BF16, name="kt")
                        nc.sync.dma_start(kt[:], qk_d[8 + h][:])
                        va = p2["va"].tile([128, NTT, 129], BF16, name="va")
                        nc.scalar.dma_start(va[:, :, 0:128], v_d[h][:])
                        nc.vector.memset(va[:, :, 128:129], 1.0)
                        yst = p2["yst"].tile([128, NTT, 128], BF16,
                                             name="yst")
                        st[h] = (qt, kt, va, yst, [])
                    return go

                def chunk(h, Q):
                    def go():
                        qt, kt, va, yst, ebs = st[h]
                        del ebs[:]
                        for b2 in range(2 * Q + 2):
                            sp = p2["sp"].tile([128, 2, 512], F32, name="sp")
                            for jj in range(2):
                                j = 2 * b2 + jj
                                nc.tensor.matmul(
                                    sp[:, jj, :],
                                    kt[:, j * 128:(j + 1) * 128],
                                    qt[:, Q * 512:(Q + 1) * 512],
                                    start=True, stop=True)
                            eb = p2["eb"].tile([128, 2, 512], BF16, name="eb")
                            nc.scalar.activation(
                                eb[:], sp[:], AF.Exp, scale=float(SCALE))
                            if b2 == 2 * Q:
                                nc.vector.tensor_mul(
                                    eb[:], eb[:], m4_sb[:, 0:2, :])
                            elif b2 == 2 * Q + 1:
                                nc.vector.tensor_mul(
                                    eb[:], eb[:], m4_sb[:, 2:4, :])
                            ebs.append(eb)
                        for ql in range(4):
                            qt_i = Q * 4 + ql
                            yp = p2["yp"].tile([128, 129], F32, name="yp")
                            for j in range(qt_i + 1):
                                nc.tensor.matmul(
                                    yp[:],
                                    ebs[j // 2][:, j % 2,
                                                ql * 128:(ql + 1) * 128],
                                    va[:, j, :],
                                    start=(j == 0), stop=(j == qt_i))
                            rc = p2["rc"].tile([128, 1], F32, name="rc")
                            nc.vector.reciprocal(rc[:], yp[:, 128:129])
                            yn = p2["yn"].tile([128, 128], BF16, name="yn")
                            nc.vector.tensor_scalar_mul(
                                yn[:], yp[:, 0:128], rc[:])
                            ytp = p2["yp"].tile([128, 128], BF16, name="ytp",
                                                tag="yp")
                            nc.tensor.transpose(ytp[:], yn[:], id_sb[:])
                            nc.vector.tensor_copy(yst[:, qt_i, :], ytp[:])
                    return go

                def epilogue(h):
                    def go():
                        yst = st[h][3]
                        nc.scalar.dma_start(
                            y_raw[h].rearrange("d (tt t) -> d tt t", t=128),
                            yst[:])
                        nc.gpsimd.collective_compute(
                            "AllGather", ALU.bypass,
                            ins=[y_raw[h][:].opt()], outs=[yg_out[h][:].opt()],
                            replica_groups=RG)
                        if emit_yf:
                            emit_yf["fn"](h)
                        del st[h]
                    return go

                for h in G:
                    units.append(prologue(h))
                    for Q in range(4):
                        units.append(chunk(h, Q))
                    units.append(epilogue(h))
                return units

            # ---------- emit ----------
            for u in p1_units(0):
                u()
            _interleave(p1_units(1), p2_units(0))

            # x + phase-1 pools done after p1 group 1; free them (LIFO) so
            # the phase-4 weight/y buffers can be allocated (right side) and
            # their DMAs issued while phase-2 group 1 computes.
            for key in ("rope", "x", "ps", "v", "qr", "ab", "w"):
                p1[key].release()

            p4w = tc.alloc_tile_pool(name="p4w", bufs=1, side="right")
            p4y = tc.alloc_tile_pool(name="p4y", bufs=1, side="right")
            pa_pool = tc.alloc_tile_pool(name="p4pa", bufs=1, side="right")
            ppa_pool = tc.alloc_tile_pool(name="p4psA", bufs=2, side="right",
                                          space="PSUM")
            wp = p4w.tile([128, NCT, C // 2], BF16)
            nc.gpsimd.dma_start(wp[:], WpT)
            partial = pa_pool.tile([128, NTT, 2, 512], BF16)
            yfs = {}

            def _emit_yf(h):
                yf = p4y.tile([128, 2, NTT, 128], BF16, name=f"yf{h}")
                eng = nc.sync if h % 2 == 0 else nc.scalar
                eng.dma_start(
                    yf[:], yg_out[h][:].rearrange(
                        "r d (tt t) -> d r tt t", t=128))
                yfs[h] = yf

            # group-0 heads have already gathered; group-1 heads emit their
            # yf load inside the epilogue, right after their collective.
            for h in GROUPS[0]:
                _emit_yf(h)
            emit_yf["fn"] = _emit_yf

            # ---------- phase 4 pass A0 (heads 0-3): interleaved with
            # phase-2 group 1 to fill its pipeline bubbles ----------
            def a0_unit(tt, fc):
                def go():
                    pp = ppa_pool.tile([128, 512], F32, name="ppA")
                    for i, (r, h) in enumerate(
                            [(r, h) for h in range(4) for r in range(2)]):
                        nc.tensor.matmul(
                            pp[:], yfs[h][:, r, tt, :],
                            wp[:, r * 8 + h, fc * 512:(fc + 1) * 512],
                            start=(i == 0), stop=(i == 7))
                    nc.vector.tensor_copy(partial[:, tt, fc, :], pp[:])
                return go

            a0_units = [a0_unit(tt, fc) for tt in range(NTT)
                        for fc in range(2)]
            _interleave(p2_units(1), a0_units)

            for key in ("yp", "sp", "rc", "yn", "yst", "eb", "va", "qk"):
                p2[key].release()

            # ---------- phase 4: pass A1 (heads 4-6) + pass B (head 7) ----
            ppb_pool = tc.alloc_tile_pool(name="p4psB", bufs=4, side="right",
                                          space="PSUM")
            o_pool = tc.alloc_tile_pool(name="p4o", bufs=4, side="right")
            for tt in range(NTT):
                for fc in range(2):
                    pp = ppb_pool.tile([128, 512], F32, name="ppB")
                    for i, (r, h) in enumerate(
                            [(r, h) for h in (4, 5, 6) for r in range(2)]):
                        nc.tensor.matmul(
                            pp[:], yfs[h][:, r, tt, :],
                            wp[:, r * 8 + h, fc * 512:(fc + 1) * 512],
                            start=(i == 0), stop=(i == 5))
                    nc.vector.tensor_add(
                        partial[:, tt, fc, :], pp[:], partial[:, tt, fc, :])
            for tt in range(NTT):
                for fc in range(2):
                    pp = ppb_pool.tile([128, 512], F32, name="ppB")
                    for i, r in enumerate((0, 1)):
                        nc.tensor.matmul(
                            pp[:], yfs[7][:, r, tt, :],
                            wp[:, r * 8 + 7, fc * 512:(fc + 1) * 512],
                            start=(i == 0), stop=(i == 1))
                    ob = o_pool.tile([128, 512], BF16, name="ob")
                    nc.vector.tensor_add(
                        ob[:], pp[:], partial[:, tt, fc, :])
                    (nc.sync if (tt + fc) % 2 == 0 else nc.scalar).dma_start(
                        out[tt * 128:(tt + 1) * 128,
                            fc * 512:(fc + 1) * 512], ob[:])

            o_pool.release()
            ppb_pool.release()
            ppa_pool.release()
            pa_pool.release()
            p4y.release()
            p4w.release()
    nc.compile()
    return nc


_NC = None


def _get_nc():
    global _NC
    if _NC is None:
        _NC = _build()
    return _NC


def _rope_tables():
    inv_freq = (1.0 / (10000.0 ** (np.arange(0, D, 2, dtype=np.float32) / D)))
    t = np.arange(T, dtype=np.float32)
    freqs = np.outer(t, inv_freq).astype(np.float32)      # [T, 64]
    cos = np.cos(freqs).T                                 # [64, T]
    sin = np.sin(freqs).T
    cos2 = np.concatenate([cos, cos], 0).astype(np.float32)
    sin1 = np.ascontiguousarray(sin.astype(np.float32))
    return cos2, sin1


def _tile_w(Wt):
    """[128 r, 2048 c] weight tile -> [128 c_lo, 2048 (ct r)] layout."""
    return np.ascontiguousarray(
        Wt.T.reshape(NCT, 128, 128).transpose(1, 0, 2).reshape(128, C))


def make_in_maps(x, W_attn, W_proj):
    perm = np.concatenate([np.arange(0, D, 2), np.arange(1, D, 2)])
    cos2, sin1 = _rope_tables()
    sgn = np.concatenate([-np.ones((64, 1)), np.ones((64, 1))]).astype(np.float32)
    p_i = np.arange(128)[:, None, None]
    jj_i = np.arange(4)[None, :, None]
    c_i = np.arange(512)[None, None, :]
    mask4 = (c_i >= p_i + 128 * jj_i).astype(BF16NP)

    in_maps = []
    for core in range(8):
        b, g = core // 2, core % 2
        tiles = []
        for G in GROUPS:
            for h in G:
                hg = g * HL + h
                tiles.append(_tile_w(W_attn[2 * C + hg * D:2 * C + (hg + 1) * D]))
            for h in G:
                hg = g * HL + h
                tiles.append(_tile_w(W_attn[hg * D:(hg + 1) * D][perm]))
                tiles.append(_tile_w(W_attn[C + hg * D:C + (hg + 1) * D][perm]))
        Wall = np.stack(tiles, 0).astype(BF16NP)
        WpT = np.ascontiguousarray(
            W_proj[g * (C // 2):(g + 1) * (C // 2), :].T
        ).reshape(NCT, 128, C // 2).transpose(1, 0, 2)
        xT8 = np.ascontiguousarray(
            x[b].T.reshape(NCT, 128, T).transpose(1, 0, 2)).astype(BF16NP)
        in_maps.append({
            "xT8": xT8,
            "Wall": Wall,
            "WpT": np.ascontiguousarray(WpT).astype(BF16NP),
            "cos2": cos2, "sin1": sin1, "sgn": sgn,
            "mask4": mask4, "ident": np.eye(128, dtype=BF16NP),
        })
    return in_maps


def _assemble(results):
    out = np.empty((B, T, C), dtype=np.float32)
    for core in range(8):
        b, g = core // 2, core % 2
        out[b][:, g * (C // 2):(g + 1) * (C // 2)] = results[core]["out"]
    return out


def run(x, W_attn, W_proj, **spmd_kwargs):
    nc = _get_nc()
    in_maps = make_in_maps(np.asarray(x, dtype=np.float32),
                           np.asarray(W_attn, dtype=np.float32),
                           np.asarray(W_proj, dtype=np.float32))
    res = bass_utils.run_bass_kernel_spmd(
        nc, in_maps, core_ids=list(range(8)), **spmd_kwargs)
    return _assemble(res.results), res


def kernel(x, W_attn, W_proj):
    out, _ = run(x, W_attn, W_proj)
    return out
